# revision 24
# baseline (speedup 1.0000x reference)
"""Trainium2 Bass kernel for nn_AnteLayer (fuzzy-rule antecedents over graph edges).

Per edge e: x1 = feat[dst,0]-feat[src,0], x2 = feat[dst,1]-feat[src,1],
ante[e, 3j+k] = exp(-2*(x1-c_j)^2) * exp(-2*(x2-c_k)^2),  c in {-1, 0, 1}.

Distribution: edge-parallel across 8 NeuronCores (800K edges each). The host
stages per-edge endpoint features (x/y planes per endpoint); each core runs a
fully pipelined streaming kernel:
  DMA-in (ACT queue) -> subtract (GPSIMD) -> 3x Derivative_Erf gaussians (ACT)
  -> 9 rule products (split DVE/GPSIMD) -> DMA-out (SP queue).

exp(-2(x-c)^2) == (sqrt(pi)/2) * Derivative_Erf(sqrt(2)*x - sqrt(2)*c), so one
ACT op per membership center; the pi/4 factor folds into the product stage.
"""
import sys

for _p in ("/opt/trn_rl_repo", "/opt/pypackages"):
    if _p not in sys.path:
        sys.path.insert(0, _p)

import math
import numpy as np

import concourse.bass as bass
import concourse.mybir as mybir
from concourse import bacc, tile
from concourse.bass_utils import run_bass_kernel_spmd

N_CORES = 8
N_EDGES = 6400000
P = 128                       # SBUF partitions
E_CORE = N_EDGES // N_CORES   # 800000 edges per core
R = E_CORE // P               # 6250 edges per partition
T = 625                       # edges per partition per tile
NT = R // T                   # tiles per core

MF_CENTERS = (-1.0, 0.0, 1.0)
SQRT2 = math.sqrt(2.0)
PI_4 = math.pi / 4.0
N_GP_PLANES = 3               # rule planes computed on GPSIMD (rest on DVE)

_nc_cache = {}


def _build():
    if "nc" in _nc_cache:
        return _nc_cache["nc"]
    nc = bacc.Bacc("TRN2", target_bir_lowering=False)
    f32 = mybir.dt.float32
    f16 = mybir.dt.float16
    # [2, P, R]: x-plane then y-plane, per endpoint (fp16 halves input traffic)
    s_ext = nc.declare_dram_parameter("xy_src", [2, P, R], f16, isOutput=False)
    d_ext = nc.declare_dram_parameter("xy_dst", [2, P, R], f16, isOutput=False)
    out_ext = nc.declare_dram_parameter("out", [P, R, 9], f32, isOutput=True)

    with tile.TileContext(nc) as tc:
        with (
            tc.tile_pool(name="consts", bufs=1) as consts,
            tc.tile_pool(name="mid", bufs=3) as mid,
            tc.tile_pool(name="oute", bufs=4) as oute,
        ):
            bias_aps = []
            for ci, c in enumerate(MF_CENTERS):
                b = consts.tile([P, 1], f32, tag=f"bias{ci}")
                nc.vector.memset(b[:, :], -SQRT2 * c)
                bias_aps.append(b)
            for it in range(NT):
                sl = slice(it * T, (it + 1) * T)
                # X = (-src) then += dst, subtract fused into the DMA (CCE add);
                # host supplies xy_src pre-negated.
                x = mid.tile([P, 2, T], f16, tag="x")
                for m in range(2):
                    nc.sync.dma_start(out=x[:, m, :], in_=s_ext[m, :, sl])
                for m in range(2):
                    nc.gpsimd.dma_start(
                        out=x[:, m, :], in_=d_ext[m, :, sl],
                        accum_op=mybir.AluOpType.add,
                    )

                # D[p,c,m,:] = Derivative_Erf(sqrt2*X - sqrt2*center_c), contiguous
                d = mid.tile([P, 3, 2, T], f32, tag="d")
                for ci in range(3):
                    nc.scalar.activation(
                        d[:, ci, :, :],
                        x[:, :, :],
                        mybir.ActivationFunctionType.Derivative_Erf,
                        bias=bias_aps[ci][:, :],
                        scale=SQRT2,
                    )

                # ante[p,t,3j+k] = (pi/4) * D[p,j,0,t] * D[p,k,1,t] -- one DVE op
                # via broadcast APs over dims [p, t, j, k]
                ante = oute.tile([P, T, 9], f32, tag="ante")
                d_full = d[:, :, :, :]
                a_full = ante[:, :, :]
                for j in range(3):
                    dx_ap = bass.AP(
                        d_full.tensor, d_full.offset + j * 2 * T,
                        [[6 * T, P], [1, T], [0, 3]],
                    )
                    dy_ap = bass.AP(
                        d_full.tensor, d_full.offset + T,
                        [[6 * T, P], [1, T], [2 * T, 3]],
                    )
                    out_ap = bass.AP(
                        a_full.tensor, a_full.offset + 3 * j,
                        [[9 * T, P], [9, T], [1, 3]],
                    )
                    nc.vector.scalar_tensor_tensor(
                        out_ap, dx_ap, PI_4, dy_ap,
                        op0=mybir.AluOpType.mult,
                        op1=mybir.AluOpType.mult,
                    )

                nc.sync.dma_start(out=out_ext[:, sl, :], in_=ante[:, :, :])

    nc.compile()
    _nc_cache["nc"] = nc
    return nc


def _shard_host(feat2, idx_shard, negate=False):
    # [2, P, R] plane-separated gathered coordinates, fp16 on the wire
    g = feat2[idx_shard]                      # [E_CORE, 2]
    g = g.reshape(P, R, 2).transpose(2, 0, 1).astype(np.float16)
    if negate:
        g = -g
    return np.ascontiguousarray(g)


def kernel(feat, edge_src, edge_dst, etypes):
    feat = np.asarray(feat, dtype=np.float32)
    edge_src = np.asarray(edge_src, dtype=np.int32)
    edge_dst = np.asarray(edge_dst, dtype=np.int32)
    del etypes  # unused by the reference computation

    nc = _build()

    feat2 = np.ascontiguousarray(feat[:, :2])  # only coords participate
    in_maps = []
    for c in range(N_CORES):
        sl = slice(c * E_CORE, (c + 1) * E_CORE)
        in_maps.append({
            "xy_src": _shard_host(feat2, edge_src[sl], negate=True),
            "xy_dst": _shard_host(feat2, edge_dst[sl]),
        })

    res = run_bass_kernel_spmd(nc, in_maps, core_ids=list(range(N_CORES)))
    out = np.empty((N_EDGES, 9), dtype=np.float32)
    for c in range(N_CORES):
        out[c * E_CORE:(c + 1) * E_CORE] = res.results[c]["out"].reshape(E_CORE, 9)
    return out
